# revision 25
# baseline (speedup 1.0000x reference)
"""Trainium2 bilateral-slice kernel (HDRNet bilateral_slice), 8-core SPMD.

ReLU-basis reformulation: the z tent weights wd_d(gz) are an exact linear
combination of {1, rho_0..rho_6}, rho_k(x) = relu(x - k - 0.5), on the
clipped domain gz in [0, 7.5]:
    wd_0 = 1 - rho_0 + rho_1 ;  wd_d = rho_{d-1} - 2 rho_d + rho_{d+1}.
Folding the x-weights wx_i in, each pixel's 16-tap weight vector
psi[(i,j)] = wx_i * phi_j(gz) is relu() of a LINEAR function of
(gz*wx_i, wx_i)  ->  one PE "broadcast" matmul produces B' in PSUM and one
fused relu pass (split DVE/ACT/Pool by columns) materializes psi fp16.
The linear combination L is folded into the stationary (host side).

Main matmul: 8 rows packed per column (K = 128 = 8 rows x 16 taps,
M = 96 = 8 rows x 12 channels).  The block-diagonal stationaries live in
SBUF with resident zeros; the nonzero [16 x (15A x 12c)] sections are
filled per u-group by a single 3-dim DMA whose leading dim strides
diagonally (16 partitions + 256 cols at once).

Sharding: core k = image k//2, h-rows [512*(k%2), 512*(k%2)+512).
Device output [u:64, (12*r8+c):96, w:1024] fp16; host reassembles + casts.
"""

import numpy as np

N_IMG, GH, GW, GD, C = 4, 16, 16, 8, 12
H = W = 1024
N_CORES = 8
ROWS_PER_CORE = 512
NU = 64                 # u-groups of 8 rows
NSLOT = 17
NA = 15                 # distinct x-corner bases a_s = clip(s-1, 0, 14)
ST_E = NA * 48          # ST cols per (u, quad): 15 A-blocks x [64, 48]
ST_HALF = 8 * ST_E      # one octave (8 u-groups) of ST data
PSI_RING = 4
CM_E = 16 * 1024        # chunk-mega free extent (16 chunks x 1024 w)


def _axis_corners(P, G):
    x = (np.arange(P) + 0.5) * (G / P)
    f = np.floor(x - 0.5)
    w1 = (x - 0.5 - f).astype(np.float64)
    w0 = 1.0 - w1
    c0 = np.clip(f, 0, G - 1).astype(np.int64)
    c1 = np.clip(f + 1, 0, G - 1).astype(np.int64)
    blk = P // G
    a = np.clip((np.arange(P) - blk // 2) // blk, 0, G - 2).astype(np.int64)
    wA = w0 * (c0 == a) + w1 * (c1 == a)
    wB = w0 * (c0 == a + 1) + w1 * (c1 == a + 1)
    return a, wA, wB


def _slot_ranges():
    return [(0, 32)] + [(64 * s - 32, 64 * s + 32) for s in range(1, 16)] + [(992, 1024)]


def _lc():
    # Lc[j, d]: wd_d = sum_j Lc[j, d] * phi_j ; phi_0 = 1, phi_j = rho_{j-1}
    Lc = np.zeros((8, 8), np.float64)
    Lc[0, 0] = 1.0
    Lc[1, 0] = -1.0
    Lc[2, 0] = 1.0
    for d in range(1, 8):
        Lc[d, d] = 1.0          # rho_{d-1} at j = d
        if d + 1 <= 7:
            Lc[d + 1, d] = -2.0
        if d + 2 <= 7:
            Lc[d + 2, d] = 1.0
    return Lc


_AX = None


def _tables():
    global _AX
    if _AX is None:
        ax, wxA, wxB = _axis_corners(W, GW)
        ay, wyA, wyB = _axis_corners(H, GH)
        _AX = (ax, wxA, wxB, ay, wyA, wyB, _lc())
    return _AX


def _consts():
    """wxpair [2, 1024] fp16 and SS [128, 128] fp16 (same for all cores)."""
    _, wxA, wxB, _, _, _, _ = _tables()
    wxpair = np.stack([wxA, wxB]).astype(np.float16)
    SS = np.zeros((128, 128), np.float16)
    for z in range(4):
        for r8 in range(8):
            for i in range(2):
                for j in range(8):
                    p = 16 * r8 + 8 * i + j
                    if j == 0:
                        SS[32 * z + 16 + i, p] = 1.0
                    else:
                        SS[32 * z + 8 * i + r8, p] = 1.0
                        SS[32 * z + 16 + i, p] = -(j - 0.5)
    return wxpair, SS


def _core_inputs(grid, guide, core):
    n = core // 2
    h0 = ROWS_PER_CORE * (core % 2)
    rows = np.arange(h0, h0 + ROWS_PER_CORE)
    _, wxA, wxB, ay, wyA, wyB, Lc = _tables()

    # gz * wx, fp16, top-clipped (bottom clamp is exact in the rho basis)
    gz = np.minimum(8.0 * guide[n, h0:h0 + ROWS_PER_CORE].astype(np.float64), 7.5)
    gzwxA = (gz * wxA[None, :]).astype(np.float16)
    gzwxB = (gz * wxB[None, :]).astype(np.float16)

    # y-interp + L-combine: T2[row, g, j, c]
    g32 = grid[n].astype(np.float64)                      # [GH, GW, GD, C]
    gy = (wyA[rows, None, None, None] * g32[ay[rows]]
          + wyB[rows, None, None, None] * g32[ay[rows] + 1])   # [512, GW, GD, C]
    T2 = np.einsum('jd,rgdc->rgjc', Lc, gy)               # [512, GW, 8, C]

    # TSQ[u, q, 16r4+8i+j, 48A + 12r4 + c] = T2[8u+4q+r4, A+i, j, c]
    TSQ = np.zeros((NU, 2, 64, NA, 48), np.float64)
    T2r = T2.reshape(NU, 2, 4, GW, 8, C)    # [u, q, r4, gw, j, c]
    for r4 in range(4):
        for i in (0, 1):
            for j in range(8):
                vals = T2r[:, :, r4, i:i + NA, j, :]      # [NU, 2, A, c]
                TSQ[:, :, 16 * r4 + 8 * i + j, :, 12 * r4:12 * r4 + C] = vals
    TSQ = TSQ.reshape(NU, 2, 64, NA * 48).astype(np.float16)

    return {"gzwxA": gzwxA, "gzwxB": gzwxB, "tsh": TSQ}


def _in_maps(grid, guide):
    grid = np.asarray(grid)
    guide = np.asarray(guide)
    wxpair, SS = _consts()
    maps = []
    for k in range(N_CORES):
        m = _core_inputs(grid, guide, k)
        m["wxp"] = wxpair
        m["ss"] = SS
        maps.append(m)
    return maps


_MODULE = None


def _build_module():
    import concourse.bacc as bacc
    import concourse.bass as bass
    import concourse.mybir as mybir
    import concourse.tile as tile

    f16, f32 = mybir.dt.float16, mybir.dt.float32
    nc = bacc.Bacc("TRN2", target_bir_lowering=False, debug=False,
                   num_devices=N_CORES)
    # the shadow race detector mis-models flat diagonal-stride APs (the
    # interpreter executes them correctly); deps are still tracked/scheduled
    nc.detect_race_conditions = False

    gzwxA = nc.dram_tensor("gzwxA", [ROWS_PER_CORE, W], f16, kind="ExternalInput")
    gzwxB = nc.dram_tensor("gzwxB", [ROWS_PER_CORE, W], f16, kind="ExternalInput")
    tsh = nc.dram_tensor("tsh", [NU, 2, 64, ST_E], f16, kind="ExternalInput")
    wxp = nc.dram_tensor("wxp", [2, W], f16, kind="ExternalInput")
    ssd = nc.dram_tensor("ss", [128, 128], f16, kind="ExternalInput")
    out = nc.dram_tensor("out", [NU, 112, W], f16, kind="ExternalOutput")

    # raw SBUF tensors. stm (the diag-AP target) MUST be the last accessed
    # allocation: the conflict checker flattens the diag stride into a huge
    # conservative span, so nothing may live above stm except the guard.
    cm = nc.alloc_sbuf_tensor("cm", [128, CM_E], f16)       # chunk-mega
    ss = nc.alloc_sbuf_tensor("sst", [128, 128], f16)
    psit = nc.alloc_sbuf_tensor("psit", [128, PSI_RING * W], f16)
    stm = nc.alloc_sbuf_tensor("stm", [128, 2 * ST_HALF], f16)
    stag = nc.alloc_sbuf_tensor("stag", [128, 2 * 8 * W], f16)
    # guard: the 2-level-partition out-DMA src APs flatten to a span that
    # overshoots stag by ~48*extent+W elements in the conservative checker
    nc.alloc_sbuf_tensor("stagguard", [128, 2 * 8 * W], f16)

    STM_E = 2 * ST_HALF
    rng = _slot_ranges()
    a_s = [min(max(s - 1, 0), 14) for s in range(NSLOT)]

    # column split for PSUM-source passes: GPSIMD cannot touch PSUM on HW,
    # so only DVE (~.76 col/ns) and ACT (~.87 col/ns) share them
    def col_split(total):
        d = int(total * 0.76 / 1.63)
        return [(0, d), (d, total)]

    opsr = [nc.alloc_psum_tensor(f"opsr{t}", [128, W], f32) for t in (0, 1)]
    bpsr = [nc.alloc_psum_tensor(f"bpsr{t}", [128, W], f32) for t in (0, 1)]

    with tile.TileContext(nc) as tc:
        if True:

            # ---- preamble: constants + whole-core gzwx + wx rows ----
            nc.sync.dma_start(out=ss[:, :], in_=ssd.ap())
            for t in (0, 1):
                nc.vector.memset(opsr[t][32:64, :], 0.0)
                nc.vector.memset(opsr[t][96:128, :], 0.0)
            for z in range(4):
                # gzwxA -> partitions 32z+0..8 ; gzwxB -> 32z+8..16
                for ti, ten in ((0, gzwxA), (1, gzwxB)):
                    dst = bass.AP(cm, (32 * z + 8 * ti) * CM_E,
                                  [[CM_E, 8], [1024, 16], [1, 1024]])
                    src = bass.AP(ten, z * 8 * 1024,
                                  [[1024, 8], [32768, 16], [1, 1024]])
                    nc.sync.dma_start(out=dst, in_=src)
                # wx rows -> partitions 32z+16 (wxA), 32z+17 (wxB)
                dstw = bass.AP(cm, (32 * z + 16) * CM_E,
                               [[CM_E, 2], [1024, 16], [1, 1024]])
                srcw = bass.AP(wxp, 0, [[1024, 2], [0, 16], [1, 1024]])
                nc.sync.dma_start(out=dstw, in_=srcw)

            def emit_front(u):
                z, cchunk = u % 4, u // 4
                # per-octave ST loads: one DMA per quad (q folded into
                # partitions), zeros resident in stm
                if u % 8 == 0:
                    o = u // 8
                    half = (o % 2) * ST_HALF
                    for q in range(2):
                        st_dst = bass.AP(
                            stm, 64 * q * STM_E + half,
                            [[STM_E, 64], [ST_E, 8], [1, ST_E]])
                        st_src = bass.AP(
                            tsh, (o * 8 * 2 + q) * 64 * ST_E,
                            [[ST_E, 64], [2 * 64 * ST_E, 8], [1, ST_E]])
                        nc.sync.dma_start(out=st_dst, in_=st_src)

                # broadcast matmul: B' = SS^T @ moving -> PSUM [128, 1024]
                bps = bpsr[u % 2]
                for h in (0, 512):
                    nc.tensor.matmul(
                        bps[:, h:h + 512],
                        ss[32 * z:32 * z + 18, 0:128],
                        cm[32 * z:32 * z + 18,
                           1024 * cchunk + h:1024 * cchunk + h + 512],
                        tile_position=(32 * z, 0),
                    )
                # psi = relu(B') fp16, split DVE / ACT by columns
                psi = psit[:, (u % PSI_RING) * W:(u % PSI_RING) * W + W]
                (d0, d1), (a0, a1) = col_split(W)
                nc.vector.tensor_scalar(psi[:, d0:d1], bps[:, d0:d1], 0.0,
                                        None, mybir.AluOpType.max)
                nc.scalar.activation(psi[:, a0:a1], bps[:, a0:a1],
                                     mybir.ActivationFunctionType.Relu)

            def emit_mid(u):
                psi = psit[:, (u % PSI_RING) * W:(u % PSI_RING) * W + W]
                # main matmuls: 17 slots x 2 quads; q0 -> rows [0:48],
                # q1 -> [64:112]
                ops = opsr[u % 2]
                st_off = (u // 8 % 2) * ST_HALF + (u % 8) * ST_E
                for s in range(NSLOT):
                    wlo, whi = rng[s]
                    spans = [(wlo, whi)] if not (wlo < 512 < whi) else \
                        [(wlo, 512), (512, whi)]
                    for q in range(2):
                        sta = bass.AP(stm, 64 * q * STM_E + st_off + 48 * a_s[s],
                                      [[STM_E, 64], [1, 48]])
                        for lo, hi in spans:
                            nc.tensor.matmul(ops[64 * q:64 * q + 48, lo:hi],
                                             sta, psi[64 * q:64 * q + 64, lo:hi],
                                             tile_position=(64 * q, 64 * q))

            def emit_back(u):
                (d0, d1), (a0, a1) = col_split(W)
                ops = opsr[u % 2]
                # stage copy (fp16), split engines by columns
                stage = stag[0:128, ((u // 8) % 2) * 8 * W:((u // 8) % 2 + 1) * 8 * W]
                w0 = (u % 8) * W
                nc.vector.tensor_copy(stage[0:112, w0 + d0:w0 + d1],
                                      ops[0:112, d0:d1])
                nc.scalar.copy(stage[0:112, w0 + a0:w0 + a1],
                               ops[0:112, a0:a1])

                # out DMA per 4 u: plain [0:112] rows (junk rows shipped,
                # host drops them)
                if u % 4 == 3:
                    se = stag.shape[1]
                    u0 = u - 3
                    odst = bass.AP(out, u0 * 112 * W,
                                   [[W, 112], [112 * W, 4], [1, W]])
                    osrc = bass.AP(stag, ((u // 8) % 2) * 8 * W + (u0 % 8) * W,
                                   [[se, 112], [W, 4], [1, W]])
                    nc.sync.dma_start(out=odst, in_=osrc)

            # software-pipelined emission: front(u) ahead of work of u-1
            for u in range(NU):
                emit_front(u)
                if u >= 1:
                    emit_mid(u - 1)
                    emit_back(u - 1)
            emit_mid(NU - 1)
            emit_back(NU - 1)
    nc.compile()
    return nc


def _get_module():
    global _MODULE
    if _MODULE is None:
        _MODULE = _build_module()
    return _MODULE


def kernel(grid, guide, trace=False, trace_kwargs=None):
    from concourse.bass_utils import run_bass_kernel_spmd

    grid = np.asarray(grid)
    guide = np.asarray(guide)
    nc = _get_module()
    in_maps = _in_maps(grid, guide)

    res = run_bass_kernel_spmd(nc, in_maps, core_ids=list(range(N_CORES)),
                               trace=trace, **(trace_kwargs or {}))

    out = np.empty((N_IMG, H, W, C), dtype=np.float32)
    for k in range(N_CORES):
        n = k // 2
        h0 = ROWS_PER_CORE * (k % 2)
        dev = res.results[k]["out"].astype(np.float32)     # [64, 112, 1024]
        # rows {64q + 12r4 + c} real, {48:64} junk; -> image row 8u+4q+r4
        dev = np.concatenate([dev[:, 0:48], dev[:, 64:112]], axis=1)
        dev = dev.reshape(NU, 2, 4, C, W).transpose(0, 1, 2, 4, 3).reshape(
            ROWS_PER_CORE, W, C)
        out[n, h0:h0 + ROWS_PER_CORE] = dev
    kernel.last_results = res
    return out


# revision 27
# speedup vs baseline: 1.8067x; 1.8067x over previous
"""Trainium2 bilateral-slice kernel (HDRNet bilateral_slice), 8-core SPMD.

ReLU-basis reformulation: the z tent weights wd_d(gz) are an exact linear
combination of {1, rho_0..rho_6}, rho_k(x) = relu(x - k - 0.5), on the
clipped domain gz in [0, 7.5]:
    wd_0 = 1 - rho_0 + rho_1 ;  wd_d = rho_{d-1} - 2 rho_d + rho_{d+1}.
Folding the x-weights wx_i in, each pixel's 16-tap weight vector
psi[(i,j)] = wx_i * phi_j(gz) is relu() of a LINEAR function of
(gz*wx_i, wx_i)  ->  one PE "broadcast" matmul produces B' in PSUM and one
fused relu pass (split DVE/ACT/Pool by columns) materializes psi fp16.
The linear combination L is folded into the stationary (host side).

Main matmul: 8 rows packed per column (K = 128 = 8 rows x 16 taps,
M = 96 = 8 rows x 12 channels).  The block-diagonal stationaries live in
SBUF with resident zeros; the nonzero [16 x (15A x 12c)] sections are
filled per u-group by a single 3-dim DMA whose leading dim strides
diagonally (16 partitions + 256 cols at once).

Sharding: core k = image k//2, h-rows [512*(k%2), 512*(k%2)+512).
Device output [u:64, (12*r8+c):96, w:1024] fp16; host reassembles + casts.
"""

import numpy as np

N_IMG, GH, GW, GD, C = 4, 16, 16, 8, 12
H = W = 1024
N_CORES = 8
ROWS_PER_CORE = 512
NU = 64                 # u-groups of 8 rows
NSLOT = 17
NA = 15                 # distinct x-corner bases a_s = clip(s-1, 0, 14)
ST_E = NA * 48          # ST cols per (u, quad): 15 A-blocks x [64, 48]
ST_HALF = 8 * ST_E      # one octave (8 u-groups) of ST data
PSI_RING = 4
CM_E = 16 * 1024        # chunk-mega free extent (16 chunks x 1024 w)


def _axis_corners(P, G):
    x = (np.arange(P) + 0.5) * (G / P)
    f = np.floor(x - 0.5)
    w1 = (x - 0.5 - f).astype(np.float64)
    w0 = 1.0 - w1
    c0 = np.clip(f, 0, G - 1).astype(np.int64)
    c1 = np.clip(f + 1, 0, G - 1).astype(np.int64)
    blk = P // G
    a = np.clip((np.arange(P) - blk // 2) // blk, 0, G - 2).astype(np.int64)
    wA = w0 * (c0 == a) + w1 * (c1 == a)
    wB = w0 * (c0 == a + 1) + w1 * (c1 == a + 1)
    return a, wA, wB


def _slot_ranges():
    return [(0, 32)] + [(64 * s - 32, 64 * s + 32) for s in range(1, 16)] + [(992, 1024)]


def _lc():
    # Lc[j, d]: wd_d = sum_j Lc[j, d] * phi_j ; phi_0 = 1, phi_j = rho_{j-1}
    Lc = np.zeros((8, 8), np.float64)
    Lc[0, 0] = 1.0
    Lc[1, 0] = -1.0
    Lc[2, 0] = 1.0
    for d in range(1, 8):
        Lc[d, d] = 1.0          # rho_{d-1} at j = d
        if d + 1 <= 7:
            Lc[d + 1, d] = -2.0
        if d + 2 <= 7:
            Lc[d + 2, d] = 1.0
    return Lc


_AX = None


def _tables():
    global _AX
    if _AX is None:
        ax, wxA, wxB = _axis_corners(W, GW)
        ay, wyA, wyB = _axis_corners(H, GH)
        _AX = (ax, wxA, wxB, ay, wyA, wyB, _lc())
    return _AX


def _consts():
    """wxpair [2, 1024] fp16 and SS [128, 128] fp16 (same for all cores)."""
    _, wxA, wxB, _, _, _, _ = _tables()
    wxpair = np.stack([wxA, wxB]).astype(np.float16)
    SS = np.zeros((128, 128), np.float16)
    for z in range(4):
        for r8 in range(8):
            for i in range(2):
                for j in range(8):
                    p = 16 * r8 + 8 * i + j
                    if j == 0:
                        SS[32 * z + 16 + i, p] = 1.0
                    else:
                        SS[32 * z + 8 * i + r8, p] = 1.0
                        SS[32 * z + 16 + i, p] = -(j - 0.5)
    return wxpair, SS


def _core_inputs(grid, guide, core):
    n = core // 2
    h0 = ROWS_PER_CORE * (core % 2)
    rows = np.arange(h0, h0 + ROWS_PER_CORE)
    _, wxA, wxB, ay, wyA, wyB, Lc = _tables()

    # gz * wx, fp16, top-clipped (bottom clamp is exact in the rho basis)
    gz = np.minimum(8.0 * guide[n, h0:h0 + ROWS_PER_CORE].astype(np.float64), 7.5)
    gzwxA = (gz * wxA[None, :]).astype(np.float16)
    gzwxB = (gz * wxB[None, :]).astype(np.float16)

    # y-interp + L-combine: T2[row, g, j, c]
    g32 = grid[n].astype(np.float64)                      # [GH, GW, GD, C]
    gy = (wyA[rows, None, None, None] * g32[ay[rows]]
          + wyB[rows, None, None, None] * g32[ay[rows] + 1])   # [512, GW, GD, C]
    T2 = np.einsum('jd,rgdc->rgjc', Lc, gy)               # [512, GW, 8, C]

    # TSQ[u, q, 16r4+8i+j, 48A + 12r4 + c] = T2[8u+4q+r4, A+i, j, c]
    TSQ = np.zeros((NU, 2, 64, NA, 48), np.float64)
    T2r = T2.reshape(NU, 2, 4, GW, 8, C)    # [u, q, r4, gw, j, c]
    for r4 in range(4):
        for i in (0, 1):
            for j in range(8):
                vals = T2r[:, :, r4, i:i + NA, j, :]      # [NU, 2, A, c]
                TSQ[:, :, 16 * r4 + 8 * i + j, :, 12 * r4:12 * r4 + C] = vals
    TSQ = TSQ.reshape(NU, 2, 64, NA * 48).astype(np.float16)

    return {"gzwxA": gzwxA, "gzwxB": gzwxB, "tsh": TSQ}


def _in_maps(grid, guide):
    grid = np.asarray(grid)
    guide = np.asarray(guide)
    wxpair, SS = _consts()
    maps = []
    for k in range(N_CORES):
        m = _core_inputs(grid, guide, k)
        m["wxp"] = wxpair
        m["ss"] = SS
        maps.append(m)
    return maps


_MODULE = None


def _build_module():
    import concourse.bacc as bacc
    import concourse.bass as bass
    import concourse.mybir as mybir
    import concourse.tile as tile

    f16, f32 = mybir.dt.float16, mybir.dt.float32
    nc = bacc.Bacc("TRN2", target_bir_lowering=False, debug=False,
                   num_devices=N_CORES)
    # the shadow race detector mis-models flat diagonal-stride APs (the
    # interpreter executes them correctly); deps are still tracked/scheduled
    nc.detect_race_conditions = False

    gzwxA = nc.dram_tensor("gzwxA", [ROWS_PER_CORE, W], f16, kind="ExternalInput")
    gzwxB = nc.dram_tensor("gzwxB", [ROWS_PER_CORE, W], f16, kind="ExternalInput")
    tsh = nc.dram_tensor("tsh", [NU, 2, 64, ST_E], f16, kind="ExternalInput")
    wxp = nc.dram_tensor("wxp", [2, W], f16, kind="ExternalInput")
    ssd = nc.dram_tensor("ss", [128, 128], f16, kind="ExternalInput")
    out = nc.dram_tensor("out", [NU, 112, W], f16, kind="ExternalOutput")

    # raw SBUF tensors. stm (the diag-AP target) MUST be the last accessed
    # allocation: the conflict checker flattens the diag stride into a huge
    # conservative span, so nothing may live above stm except the guard.
    cm = nc.alloc_sbuf_tensor("cm", [128, CM_E], f16)       # chunk-mega
    ss = nc.alloc_sbuf_tensor("sst", [128, 128], f16)
    psit = nc.alloc_sbuf_tensor("psit", [128, PSI_RING * W], f16)
    stm = nc.alloc_sbuf_tensor("stm", [128, 2 * ST_HALF], f16)
    stag = nc.alloc_sbuf_tensor("stag", [128, 2 * 8 * W], f16)
    # guard: the 2-level-partition out-DMA src APs flatten to a span that
    # overshoots stag by ~48*extent+W elements in the conservative checker
    nc.alloc_sbuf_tensor("stagguard", [128, 2 * 8 * W], f16)

    STM_E = 2 * ST_HALF
    rng = _slot_ranges()
    a_s = [min(max(s - 1, 0), 14) for s in range(NSLOT)]

    # column split for PSUM-source passes: GPSIMD cannot touch PSUM on HW,
    # so only DVE (~.76 col/ns) and ACT (~.87 col/ns) share them
    def col_split(total):
        d = int(total * 0.76 / 1.63)
        return [(0, d), (d, total)]

    opsr = [nc.alloc_psum_tensor(f"opsr{t}", [128, W], f32) for t in (0, 1)]
    bpsr = [nc.alloc_psum_tensor(f"bpsr{t}", [128, W], f32) for t in (0, 1)]

    with tile.TileContext(nc) as tc:
        if True:

            # ---- preamble: constants + whole-core gzwx + wx rows ----
            nc.sync.dma_start(out=ss[:, :], in_=ssd.ap())
            for t in (0, 1):
                nc.vector.memset(opsr[t][32:64, :], 0.0)
                nc.vector.memset(opsr[t][96:128, :], 0.0)
            for z in range(4):
                # gzwxA -> partitions 32z+0..8 ; gzwxB -> 32z+8..16
                for ti, ten in ((0, gzwxA), (1, gzwxB)):
                    dst = bass.AP(cm, (32 * z + 8 * ti) * CM_E,
                                  [[CM_E, 8], [1024, 16], [1, 1024]])
                    src = bass.AP(ten, z * 8 * 1024,
                                  [[1024, 8], [32768, 16], [1, 1024]])
                    nc.sync.dma_start(out=dst, in_=src)
                # wx rows -> partitions 32z+16 (wxA), 32z+17 (wxB)
                dstw = bass.AP(cm, (32 * z + 16) * CM_E,
                               [[CM_E, 2], [1024, 16], [1, 1024]])
                srcw = bass.AP(wxp, 0, [[1024, 2], [0, 16], [1, 1024]])
                nc.sync.dma_start(out=dstw, in_=srcw)

            def emit_front(u):
                z, cchunk = u % 4, u // 4
                # per-octave ST loads: one DMA per quad (q folded into
                # partitions), zeros resident in stm
                if u % 8 == 0:
                    o = u // 8
                    half = (o % 2) * ST_HALF
                    for q in range(2):
                        st_dst = bass.AP(
                            stm, 64 * q * STM_E + half,
                            [[STM_E, 64], [ST_E, 8], [1, ST_E]])
                        st_src = bass.AP(
                            tsh, (o * 8 * 2 + q) * 64 * ST_E,
                            [[ST_E, 64], [2 * 64 * ST_E, 8], [1, ST_E]])
                        nc.sync.dma_start(out=st_dst, in_=st_src)

                # broadcast matmul: B' = SS^T @ moving -> PSUM [128, 1024]
                bps = bpsr[u % 2]
                for h in (0, 512):
                    nc.tensor.matmul(
                        bps[:, h:h + 512],
                        ss[32 * z:32 * z + 18, 0:128],
                        cm[32 * z:32 * z + 18,
                           1024 * cchunk + h:1024 * cchunk + h + 512],
                        tile_position=(32 * z, 0),
                    )
                # psi = relu(B') fp16, split DVE / ACT by columns
                psi = psit[:, (u % PSI_RING) * W:(u % PSI_RING) * W + W]
                (d0, d1), (a0, a1) = col_split(W)
                nc.vector.tensor_scalar(psi[:, d0:d1], bps[:, d0:d1], 0.0,
                                        None, mybir.AluOpType.max)
                nc.scalar.activation(psi[:, a0:a1], bps[:, a0:a1],
                                     mybir.ActivationFunctionType.Relu)

            def emit_mid(u):
                psi = psit[:, (u % PSI_RING) * W:(u % PSI_RING) * W + W]
                # main matmuls: 17 slots x 2 quads; q0 -> rows [0:48],
                # q1 -> [64:112]
                ops = opsr[u % 2]
                st_off = (u // 8 % 2) * ST_HALF + (u % 8) * ST_E
                for s in range(NSLOT):
                    wlo, whi = rng[s]
                    spans = [(wlo, whi)] if not (wlo < 512 < whi) else \
                        [(wlo, 512), (512, whi)]
                    for q in range(2):
                        sta = bass.AP(stm, 64 * q * STM_E + st_off + 48 * a_s[s],
                                      [[STM_E, 64], [1, 48]])
                        for lo, hi in spans:
                            nc.tensor.matmul(ops[64 * q:64 * q + 48, lo:hi],
                                             sta, psi[64 * q:64 * q + 64, lo:hi],
                                             tile_position=(64 * q, 64 * q))

            def emit_back(u):
                (d0, d1), (a0, a1) = col_split(W)
                ops = opsr[u % 2]
                # stage copy (fp16), split engines by columns
                stage = stag[0:128, ((u // 8) % 2) * 8 * W:((u // 8) % 2 + 1) * 8 * W]
                w0 = (u % 8) * W
                nc.vector.tensor_copy(stage[0:112, w0 + d0:w0 + d1],
                                      ops[0:112, d0:d1])
                nc.scalar.copy(stage[0:112, w0 + a0:w0 + a1],
                               ops[0:112, a0:a1])

                # out DMA per 4 u: plain [0:112] rows (junk rows shipped,
                # host drops them)
                if u % 4 == 3:
                    se = stag.shape[1]
                    u0 = u - 3
                    odst = bass.AP(out, u0 * 112 * W,
                                   [[W, 112], [112 * W, 4], [1, W]])
                    osrc = bass.AP(stag, ((u // 8) % 2) * 8 * W + (u0 % 8) * W,
                                   [[se, 112], [W, 4], [1, W]])
                    nc.sync.dma_start(out=odst, in_=osrc)

            # software-pipelined emission: front(u) ahead of work of u-1
            for u in range(NU):
                emit_front(u)
                if u >= 1:
                    emit_mid(u - 1)
                    emit_back(u - 1)
            emit_mid(NU - 1)
            emit_back(NU - 1)
    nc.compile()
    return nc


def _get_module():
    global _MODULE
    if _MODULE is None:
        _MODULE = _build_module()
    return _MODULE


def kernel(grid, guide, trace=False, trace_kwargs=None):
    from concourse.bass_utils import run_bass_kernel_spmd

    grid = np.asarray(grid)
    guide = np.asarray(guide)
    nc = _get_module()
    in_maps = _in_maps(grid, guide)

    res = run_bass_kernel_spmd(nc, in_maps, core_ids=list(range(N_CORES)),
                               trace=trace, **(trace_kwargs or {}))

    out = np.empty((N_IMG, H, W, C), dtype=np.float32)
    for k in range(N_CORES):
        n = k // 2
        h0 = ROWS_PER_CORE * (k % 2)
        dev = res.results[k]["out"].astype(np.float32)     # [64, 112, 1024]
        # rows {64q + 12r4 + c} real, {48:64} junk; -> image row 8u+4q+r4
        dev = np.concatenate([dev[:, 0:48], dev[:, 64:112]], axis=1)
        dev = dev.reshape(NU, 2, 4, C, W).transpose(0, 1, 2, 4, 3).reshape(
            ROWS_PER_CORE, W, C)
        out[n, h0:h0 + ROWS_PER_CORE] = dev
    kernel.last_results = res
    return out
